# revision 3
# baseline (speedup 1.0000x reference)
"""Trainium2 Bass kernel for Swin-style multi-head attention (bf16 v2).

Problem: x[128,197,768] -> qkv -> 12-head attention with relative-position
bias -> proj. Data-parallel over batch across 8 NeuronCores (16 batches/core).

v2 design vs the fp32r baseline:
  - All matmul operands are bf16 (1 cyc/col, same as fp32r, but with no
    >=256 free-dim requirement), so the n dimension is NOT padded to 256:
    scores/AV/exp/bias-mul process 197-wide tiles (-23% columns there).
  - Host ships xT / qkv_wT / proj_wT / exp(bias) already in bf16: DMA
    traffic halves and every fp32r staging copy (ACT+GPSIMD pairs in the
    baseline) collapses into one direct ACT copy PSUM->bf16 SBUF.
  - Epilogue balance: ACT does exp, qk/v copies and the proj bias-add
    (Identity activation + per-partition bias AP); DVE does the bias
    multiply (2x bf16 mode), reciprocal and the normalize multiply;
    GPSIMD only does one merged partition_broadcast per unit.
  - vaug tiles ([t, 12, 65] with a persistent ones column for the
    augmented-matmul rowsum trick) are allocated per pair-parity and the
    ones column is initialized once, outside the pair loop.
"""

import sys

import numpy as np

for _p in ('/opt/trn_rl_repo', '/root/.axon_site/_ro/trn_rl_repo'):
    if _p not in sys.path:
        sys.path.insert(0, _p)

B = 128
N = 197
C = 768
H = 12
DH = 64
SCALE = DH ** -0.5
NCORES = 8
BLOC = B // NCORES  # 16
M0, M1 = 128, N - 128  # key-dim tiles: 128 + 69


def build_nc(b_loc=BLOC, pdepth=4, reps=1):
    import concourse.bacc as bacc
    import concourse.tile as tile
    from concourse import library_config, mybir

    f32 = mybir.dt.float32
    bf16 = mybir.dt.bfloat16

    nc = bacc.Bacc("TRN2", target_bir_lowering=False, debug=False)
    xT = nc.dram_tensor("xT", [b_loc, C, N], bf16, kind="ExternalInput").ap()
    qkv_wT = nc.dram_tensor("qkv_wT", [C, 3 * C], bf16, kind="ExternalInput").ap()
    proj_wT = nc.dram_tensor("proj_wT", [C, C], bf16, kind="ExternalInput").ap()
    proj_bt = nc.dram_tensor("proj_bt", [128, 6], f32, kind="ExternalInput").ap()
    biasT = nc.dram_tensor("biasT", [N, H, N], bf16, kind="ExternalInput").ap()
    outT = nc.dram_tensor("outT", [b_loc, C, N], f32, kind="ExternalOutput").ap()

    n_pairs = b_loc // 2

    with tile.TileContext(nc) as tc:
        with (
            tc.tile_pool(name="consts", bufs=1) as consts,
            tc.tile_pool(name="xtp", bufs=2) as xtp,
            tc.tile_pool(name="qkp", bufs=2) as qkp,
            tc.tile_pool(name="sap", bufs=4) as sap,
            tc.tile_pool(name="vstp", bufs=2) as vstp,
            tc.tile_pool(name="pup", bufs=2 * (pdepth + 1)) as pup,
            tc.tile_pool(name="recp", bufs=3) as recp,
            tc.tile_pool(name="oallp", bufs=2) as oallp,
            tc.tile_pool(name="obp", bufs=6) as obp,
            tc.tile_pool(name="psbig", bufs=2, space="PSUM") as psbig,
            tc.tile_pool(name="pss", bufs=2, space="PSUM") as pss,
            tc.tile_pool(name="pso", bufs=2, space="PSUM") as pso,
        ):
            nc.gpsimd.load_library(library_config.attnmlp)

            qkvw_sb = consts.tile([128, 6, 3 * C], bf16)
            projw_sb = consts.tile([128, 6, C], bf16)
            projb_sb = consts.tile([128, 6], f32)
            bias0_sb = consts.tile([128, H, N], bf16)
            bias1_sb = consts.tile([128, H, N], bf16)
            bias_sb = (bias0_sb, bias1_sb)
            # vaug tiles [t, 12, 65]: per parity x batch-in-pair x t-chunk.
            # Column 64 is a persistent 1.0 (rowsum trick), set once below;
            # the v copies never touch it.
            vts_all = [
                [
                    [
                        consts.tile([128, H, 65], bf16, name=f"vt{par}{b}{tci}")
                        for tci in (0, 1)
                    ]
                    for b in (0, 1)
                ]
                for par in (0, 1)
            ]

            # split the big weight load across queues; qkv first (needed by
            # the first pair's matmuls), proj weights last (needed ~30us in)
            for ct in range(6):
                nc.sync.dma_start(
                    qkvw_sb[:, ct], qkv_wT[ct * 128:(ct + 1) * 128, :]
                )
            nc.sync.dma_start(projb_sb, proj_bt)
            nc.sync.dma_start(bias0_sb, biasT[0:M0])
            nc.sync.dma_start(bias1_sb[:M1], biasT[M0:N])
            nc.sync.dma_start(
                projw_sb, proj_wT.rearrange("(hp p) e -> p hp e", p=128)
            )
            ones_sb = consts.tile([128, H], bf16)
            nc.vector.memset(ones_sb, 1.0)
            for par in (0, 1):
                for b in (0, 1):
                    for tci in (0, 1):
                        nc.gpsimd.tensor_copy(
                            out=vts_all[par][b][tci][:, :, 64], in_=ones_sb
                        )

            def load_xt(p):
                xt = xtp.tile([128, 6, 2, N], bf16)
                for b in (0, 1):
                    nc.sync.dma_start(
                        xt[:, :, b, :],
                        xT[2 * (p % n_pairs) + b].rearrange(
                            "(ct p) n -> p ct n", p=128
                        ),
                    )
                return xt

            def make_qkv_units(xt_, qk_, vts_):
                """20 PE-burst closures: 12 qk f-tiles + 8 v chunks."""
                units = []
                for ft in range(12):
                    def qk_unit(ft=ft):
                        ps = psbig.tile([128, 2, N], f32, tag="mmbig")
                        for ct in range(6):
                            nc.tensor.matmul(
                                ps,
                                qkvw_sb[:, ct, ft * 128:(ft + 1) * 128],
                                xt_[:, ct],
                                start=(ct == 0),
                                stop=(ct == 5),
                            )
                        nc.scalar.copy(out=qk_[:, 0, ft], in_=ps[0:64])
                        nc.vector.tensor_copy(out=qk_[:, 1, ft], in_=ps[64:128])
                    units.append(qk_unit)
                for b in (0, 1):
                    for tci, (t0, tsz) in enumerate(((0, M0), (M0, M1))):
                        for half in (0, 1):
                            def v_unit(b=b, tci=tci, t0=t0, tsz=tsz, half=half):
                                vt = vts_[b][tci]
                                vt_r = vt.rearrange(
                                    "p (g two) c -> p two g c", two=2
                                )
                                psv = psbig.tile([128, 384], f32, tag="mmbig")
                                for ct in range(6):
                                    nc.tensor.matmul(
                                        psv[:tsz],
                                        xt_[:, ct, b, t0:t0 + tsz],
                                        qkvw_sb[:, ct,
                                                2 * C + half * 384:
                                                2 * C + (half + 1) * 384],
                                        start=(ct == 0),
                                        stop=(ct == 5),
                                    )
                                vst = vstp.tile([128, 384], bf16, tag="vst")
                                nc.scalar.copy(out=vst[:tsz], in_=psv[:tsz])
                                vst_r = vst.rearrange(
                                    "p (g two d) -> p two g d", two=2, d=64
                                )
                                for par in (0, 1):
                                    nc.gpsimd.tensor_copy(
                                        out=vt_r[:tsz, par,
                                                 half * 3:(half + 1) * 3, 0:64],
                                        in_=vst_r[:tsz, par],
                                    )
                            units.append(v_unit)
                return units

            prev_proj_units = []
            total_pairs = reps * n_pairs
            # pair 0's qkv runs un-interleaved (prologue)
            xt = load_xt(0)
            qk = qkp.tile([64, 2, H, 2, N], bf16)
            for u in make_qkv_units(xt, qk, vts_all[0]):
                u()
            for pp in range(total_pairs):
                b0 = 2 * (pp % n_pairs)
                parity = pp % 2
                vts = vts_all[parity]

                # next pair's inputs + qkv filler closures, interleaved into
                # this pair's attention so the PE never starves at the tail
                if pp + 1 < total_pairs:
                    next_xt = load_xt(pp + 1)
                    next_qk = qkp.tile([64, 2, H, 2, N], bf16)
                    fillers = make_qkv_units(
                        next_xt, next_qk, vts_all[(pp + 1) % 2]
                    )
                else:
                    next_xt = next_qk = None
                    fillers = []

                # ---- attention, software-pipelined over (batch, head-pair) ----
                o_all = oallp.tile([128, 6, 2, N], bf16)

                def emit_scores(b, hp):
                    h0 = 2 * hp
                    q0 = qk[:, 0, hp, b, :]
                    k0 = qk[:, 0, 6 + hp, b, :]
                    q1 = qk[:, 1, hp, b, :]
                    k1 = qk[:, 1, 6 + hp, b, :]
                    pus = []
                    for mt, (m0, msz) in enumerate(((0, M0), (M0, M1))):
                        ps_s = pss.tile([128, 2, N], f32, tag=f"s{mt}")
                        nc.tensor.matmul(
                            ps_s[:msz, 0, :], k0[:, m0:m0 + msz], q0,
                            start=True, stop=True,
                        )
                        nc.tensor.matmul(
                            ps_s[:msz, 1, :], k1[:, m0:m0 + msz], q1,
                            start=True, stop=True,
                        )
                        sa = sap.tile([128, 2, N], bf16, tag=f"sa{mt}")
                        nc.scalar.activation(
                            out=sa[:msz], in_=ps_s[:msz],
                            func=mybir.ActivationFunctionType.Exp, scale=SCALE,
                        )
                        pu = pup.tile([128, 2, N], bf16, tag=f"pu{mt}")
                        nc.vector.tensor_mul(
                            out=pu[:msz], in0=sa[:msz],
                            in1=bias_sb[mt][:msz, h0:h0 + 2, :],
                        )
                        pus.append(pu)
                    return pus

                def emit_av(b, hp, pus):
                    h0, h1 = 2 * hp, 2 * hp + 1
                    vt0, vt1 = vts[b]
                    ps_o = pso.tile([128, 2, N], f32, tag="opair")
                    for par, h in ((0, h0), (1, h1)):
                        nc.tensor.matmul(
                            ps_o[0:65, par, :], vt0[:, h, :],
                            pus[0][:, par, :], start=True, stop=False,
                        )
                        nc.tensor.matmul(
                            ps_o[0:65, par, :], vt1[:M1, h, :],
                            pus[1][:M1, par, :], start=False, stop=True,
                        )
                    rec = recp.tile([1, 2, N], f32, tag="rec")
                    nc.vector.reciprocal(out=rec, in_=ps_o[64:65, :, :])
                    recb = recp.tile([64, 2, N], f32, tag="recb")
                    nc.gpsimd.partition_broadcast(recb, rec)
                    for par in (0, 1):
                        nc.vector.tensor_mul(
                            out=o_all[par * 64:par * 64 + 64, hp, b, :],
                            in0=ps_o[0:64, par, :],
                            in1=recb[:, par, :],
                        )

                def make_proj_unit(et, o_all_=o_all, b0_=b0):
                    def unit():
                        psp = psbig.tile([128, 2, N], f32, tag="mmbig",
                                         name=f"psp{et}")
                        for hp in range(6):
                            nc.tensor.matmul(
                                psp,
                                projw_sb[:, hp, et * 128:(et + 1) * 128],
                                o_all_[:, hp],
                                start=(hp == 0),
                                stop=(hp == 5),
                            )
                        ob = obp.tile([128, 2, N], f32, tag="ob", name=f"ob{et}")
                        nc.scalar.add(ob, psp, projb_sb[:, et:et + 1])
                        nc.sync.dma_start(
                            outT[b0_:b0_ + 2, et * 128:(et + 1) * 128, :]
                            .rearrange("b p n -> p b n"),
                            ob,
                        )
                    return unit

                work = [(b, hp) for b in (0, 1) for hp in range(6)]
                pending = []
                for i, (b, hp) in enumerate(work):
                    pending.append((b, hp, emit_scores(b, hp)))
                    if prev_proj_units and i % 2 == 1:
                        prev_proj_units.pop(0)()
                    # spread next pair's 20 qkv chunks over units 2..11;
                    # delayed past unit 2 so the ACT queue doesn't head-of-
                    # line block on v-copies waiting for prev-pair AV reads
                    if i >= 2:
                        for _ in range(2):
                            if fillers:
                                fillers.pop(0)()
                    if len(pending) > pdepth:
                        pb, php, ppus = pending.pop(0)
                        emit_av(pb, php, ppus)
                for pb, php, ppus in pending:
                    emit_av(pb, php, ppus)
                while fillers:
                    fillers.pop(0)()
                for u in prev_proj_units:
                    u()
                prev_proj_units = [make_proj_unit(et) for et in range(6)]
                xt, qk = next_xt, next_qk

            # ---- final pair's proj ----
            for u in prev_proj_units:
                u()
    nc.compile()
    return nc


def _to_bf16(a):
    import jax.numpy as jnp

    return np.asarray(jnp.asarray(a, jnp.bfloat16))


def prep_inputs(x, qkv_w, proj_w, proj_b, bias_table, rel_idx):
    """Host-side data prep shared by kernel() and test harness."""
    x = np.asarray(x, np.float32)
    qkv_w = np.asarray(qkv_w, np.float32)
    proj_w = np.asarray(proj_w, np.float32)
    proj_b = np.asarray(proj_b, np.float32)
    bias_table = np.asarray(bias_table, np.float32)
    rel_idx = np.asarray(rel_idx)

    xT = np.ascontiguousarray(x.reshape(NCORES, BLOC, N, C).transpose(0, 1, 3, 2))
    qkv_wT = np.ascontiguousarray(qkv_w.T)
    proj_wT = np.ascontiguousarray(proj_w.T)
    proj_bt = np.ascontiguousarray(proj_b.reshape(6, 128).T)
    bias_full = bias_table[rel_idx]  # [n, m, h]
    # biasT[m, h, n] = exp(bias[n, m, h])
    biasT = np.ascontiguousarray(np.exp(bias_full).transpose(1, 2, 0))
    return (
        _to_bf16(xT), _to_bf16(qkv_wT), _to_bf16(proj_wT),
        proj_bt, _to_bf16(biasT),
    )


_NC_CACHE = {}


def _get_nc(**kw):
    key = tuple(sorted(kw.items()))
    if key not in _NC_CACHE:
        _NC_CACHE[key] = build_nc(**kw)
    return _NC_CACHE[key]


def kernel(x, qkv_w, proj_w, proj_b, bias_table, rel_idx, _trace=False):
    from concourse.bass_utils import run_bass_kernel_spmd

    xT, qkv_wT, proj_wT, proj_bt, biasT = prep_inputs(
        x, qkv_w, proj_w, proj_b, bias_table, rel_idx
    )
    nc = _get_nc()
    in_maps = [
        {
            "xT": np.ascontiguousarray(xT[c]),
            "qkv_wT": qkv_wT,
            "proj_wT": proj_wT,
            "proj_bt": proj_bt,
            "biasT": biasT,
        }
        for c in range(NCORES)
    ]
    res = run_bass_kernel_spmd(nc, in_maps, list(range(NCORES)), trace=_trace)
    outs = np.stack([res.results[c]["outT"] for c in range(NCORES)])
    out = outs.reshape(B, C, N).transpose(0, 2, 1)
    out = np.ascontiguousarray(out, np.float32)
    if _trace:
        return out, res
    return out


# revision 14
# speedup vs baseline: 1.2859x; 1.2859x over previous
"""Trainium2 Bass kernel for Swin-style multi-head attention (bf16 v2).

Problem: x[128,197,768] -> qkv -> 12-head attention with relative-position
bias -> proj. Data-parallel over batch across 8 NeuronCores (16 batches/core).

v2 design vs the fp32r baseline:
  - All matmul operands are bf16 (1 cyc/col, same as fp32r, but with no
    >=256 free-dim requirement), so the n dimension is NOT padded to 256:
    scores/AV/exp/bias-mul process 197-wide tiles (-23% columns there).
  - Host ships xT / qkv_wT / proj_wT / exp(bias) already in bf16: DMA
    traffic halves and every fp32r staging copy (ACT+GPSIMD pairs in the
    baseline) collapses into one direct ACT copy PSUM->bf16 SBUF.
  - Epilogue balance: ACT does exp, qk/v copies and the proj bias-add
    (Identity activation + per-partition bias AP); DVE does the bias
    multiply (2x bf16 mode), reciprocal and the normalize multiply;
    GPSIMD only does one merged partition_broadcast per unit.
  - vaug tiles ([t, 12, 65] with a persistent ones column for the
    augmented-matmul rowsum trick) are allocated per pair-parity and the
    ones column is initialized once, outside the pair loop.
"""

import sys

import numpy as np

for _p in ('/opt/trn_rl_repo', '/root/.axon_site/_ro/trn_rl_repo'):
    if _p not in sys.path:
        sys.path.insert(0, _p)

B = 128
N = 197
C = 768
H = 12
DH = 64
SCALE = DH ** -0.5
NCORES = 8
BLOC = B // NCORES  # 16
M0, M1 = 128, N - 128  # key-dim tiles: 128 + 69


def build_nc(b_loc=BLOC, pdepth=4, reps=1):
    import concourse.bacc as bacc
    import concourse.tile as tile
    from concourse import library_config, mybir

    f32 = mybir.dt.float32
    bf16 = mybir.dt.bfloat16

    nc = bacc.Bacc("TRN2", target_bir_lowering=False, debug=False)
    xT = nc.dram_tensor("xT", [b_loc, C, N], bf16, kind="ExternalInput").ap()
    qkv_wT = nc.dram_tensor("qkv_wT", [C, 3 * C], bf16, kind="ExternalInput").ap()
    proj_wT = nc.dram_tensor("proj_wT", [C, C], bf16, kind="ExternalInput").ap()
    proj_bt = nc.dram_tensor("proj_bt", [128, 6], f32, kind="ExternalInput").ap()
    biasT = nc.dram_tensor("biasT", [N, H, N], bf16, kind="ExternalInput").ap()
    outT = nc.dram_tensor("outT", [b_loc, C, N], f32, kind="ExternalOutput").ap()

    n_pairs = b_loc // 2

    with tile.TileContext(nc) as tc:
        with (
            tc.tile_pool(name="consts", bufs=1) as consts,
            tc.tile_pool(name="xtp", bufs=2) as xtp,
            tc.tile_pool(name="qkp", bufs=2) as qkp,
            tc.tile_pool(name="sap", bufs=4) as sap,
            tc.tile_pool(name="vstp", bufs=2) as vstp,
            tc.tile_pool(name="pup", bufs=2 * (pdepth + 1)) as pup,
            tc.tile_pool(name="recp", bufs=3) as recp,
            tc.tile_pool(name="oallp", bufs=2) as oallp,
            tc.tile_pool(name="obp", bufs=6) as obp,
            tc.tile_pool(name="psbig", bufs=2, space="PSUM") as psbig,
            tc.tile_pool(name="pss", bufs=2, space="PSUM") as pss,
            tc.tile_pool(name="pso", bufs=2, space="PSUM") as pso,
        ):
            nc.gpsimd.load_library(library_config.attnmlp)

            qkvw_sb = consts.tile([128, 6, 3 * C], bf16)
            projw_sb = consts.tile([128, 6, C], bf16)
            projb_sb = consts.tile([128, 6], f32)
            bias0_sb = consts.tile([128, H, N], bf16)
            bias1_sb = consts.tile([128, H, N], bf16)
            bias_sb = (bias0_sb, bias1_sb)
            # vaug tiles [t, 12, 65]: per parity x batch-in-pair x t-chunk.
            # Column 64 is a persistent 1.0 (rowsum trick), set once below;
            # the v copies never touch it.
            vts_all = [
                [
                    [
                        consts.tile([128, H, 65], bf16, name=f"vt{par}{b}{tci}")
                        for tci in (0, 1)
                    ]
                    for b in (0, 1)
                ]
                for par in (0, 1)
            ]

            # split the big weight load across queues; qkv first (needed by
            # the first pair's matmuls), proj weights last (needed ~30us in)
            for ct in range(6):
                nc.sync.dma_start(
                    qkvw_sb[:, ct], qkv_wT[ct * 128:(ct + 1) * 128, :]
                )
            nc.sync.dma_start(projb_sb, proj_bt)
            nc.sync.dma_start(bias0_sb, biasT[0:M0])
            nc.sync.dma_start(bias1_sb[:M1], biasT[M0:N])
            nc.sync.dma_start(
                projw_sb, proj_wT.rearrange("(hp p) e -> p hp e", p=128)
            )
            ones_sb = consts.tile([128, H], bf16)
            nc.vector.memset(ones_sb, 1.0)
            for par in (0, 1):
                for b in (0, 1):
                    for tci in (0, 1):
                        nc.gpsimd.tensor_copy(
                            out=vts_all[par][b][tci][:, :, 64], in_=ones_sb
                        )

            def load_xt(p):
                xt = xtp.tile([128, 6, 2, N], bf16)
                for b in (0, 1):
                    nc.sync.dma_start(
                        xt[:, :, b, :],
                        xT[2 * (p % n_pairs) + b].rearrange(
                            "(ct p) n -> p ct n", p=128
                        ),
                    )
                return xt

            def make_qkv_units(xt_, qk_, vts_):
                """20 PE-burst closures: 12 qk f-tiles + 8 v chunks."""
                units = []
                for ft in range(12):
                    def qk_unit(ft=ft):
                        ps = psbig.tile([128, 2, N], f32, tag="mmbig")
                        for ct in range(6):
                            nc.tensor.matmul(
                                ps,
                                qkvw_sb[:, ct, ft * 128:(ft + 1) * 128],
                                xt_[:, ct],
                                start=(ct == 0),
                                stop=(ct == 5),
                            )
                        nc.scalar.copy(out=qk_[:, 0, ft], in_=ps[0:64])
                        nc.vector.tensor_copy(out=qk_[:, 1, ft], in_=ps[64:128])
                    units.append(qk_unit)
                for b in (0, 1):
                    for tci, (t0, tsz) in enumerate(((0, M0), (M0, M1))):
                        for half in (0, 1):
                            def v_unit(b=b, tci=tci, t0=t0, tsz=tsz, half=half):
                                vt = vts_[b][tci]
                                vt_r = vt.rearrange(
                                    "p (g two) c -> p two g c", two=2
                                )
                                psv = psbig.tile([128, 384], f32, tag="mmbig")
                                for ct in range(6):
                                    nc.tensor.matmul(
                                        psv[:tsz],
                                        xt_[:, ct, b, t0:t0 + tsz],
                                        qkvw_sb[:, ct,
                                                2 * C + half * 384:
                                                2 * C + (half + 1) * 384],
                                        start=(ct == 0),
                                        stop=(ct == 5),
                                    )
                                nc.scalar.copy(
                                    out=vt_r[:tsz, :,
                                             half * 3:(half + 1) * 3, 0:64]
                                    .rearrange("p two g d -> p g two d"),
                                    in_=psv[:tsz].rearrange(
                                        "p (g two d) -> p g two d", g=3, two=2
                                    ),
                                )
                            units.append(v_unit)
                return units

            prev_proj_units = []
            total_pairs = reps * n_pairs
            # pair 0's qkv runs un-interleaved (prologue)
            xt = load_xt(0)
            qk = qkp.tile([64, 2, H, 2, N], bf16)
            for u in make_qkv_units(xt, qk, vts_all[0]):
                u()
            for pp in range(total_pairs):
                b0 = 2 * (pp % n_pairs)
                parity = pp % 2
                vts = vts_all[parity]

                # next pair's inputs + qkv filler closures, interleaved into
                # this pair's attention so the PE never starves at the tail
                if pp + 1 < total_pairs:
                    next_xt = load_xt(pp + 1)
                    next_qk = qkp.tile([64, 2, H, 2, N], bf16)
                    fillers = make_qkv_units(
                        next_xt, next_qk, vts_all[(pp + 1) % 2]
                    )
                else:
                    next_xt = next_qk = None
                    fillers = []

                # ---- attention, software-pipelined over (batch, head-pair) ----
                o_all = oallp.tile([128, 6, 2, N], bf16)

                def emit_scores(b, hp):
                    h0 = 2 * hp
                    q0 = qk[:, 0, hp, b, :]
                    k0 = qk[:, 0, 6 + hp, b, :]
                    q1 = qk[:, 1, hp, b, :]
                    k1 = qk[:, 1, 6 + hp, b, :]
                    pus = []
                    for mt, (m0, msz) in enumerate(((0, M0), (M0, M1))):
                        ps_s = pss.tile([128, 2, N], f32, tag=f"s{mt}")
                        nc.tensor.matmul(
                            ps_s[:msz, 0, :], k0[:, m0:m0 + msz], q0,
                            start=True, stop=True,
                        )
                        nc.tensor.matmul(
                            ps_s[:msz, 1, :], k1[:, m0:m0 + msz], q1,
                            start=True, stop=True,
                        )
                        sa = sap.tile([128, 2, N], bf16, tag=f"sa{mt}")
                        nc.scalar.activation(
                            out=sa[:msz], in_=ps_s[:msz],
                            func=mybir.ActivationFunctionType.Exp, scale=SCALE,
                        )
                        pu = pup.tile([128, 2, N], bf16, tag=f"pu{mt}")
                        nc.vector.tensor_mul(
                            out=pu[:msz], in0=sa[:msz],
                            in1=bias_sb[mt][:msz, h0:h0 + 2, :],
                        )
                        pus.append(pu)
                    return pus

                def emit_av(b, hp, pus):
                    h0, h1 = 2 * hp, 2 * hp + 1
                    vt0, vt1 = vts[b]
                    ps_o = pso.tile([128, 2, N], f32, tag="opair")
                    for par, h in ((0, h0), (1, h1)):
                        nc.tensor.matmul(
                            ps_o[0:65, par, :], vt0[:, h, :],
                            pus[0][:, par, :], start=True, stop=False,
                        )
                        nc.tensor.matmul(
                            ps_o[0:65, par, :], vt1[:M1, h, :],
                            pus[1][:M1, par, :], start=False, stop=True,
                        )
                    rec = recp.tile([1, 2, N], f32, tag="rec")
                    nc.vector.reciprocal(out=rec, in_=ps_o[64:65, :, :])
                    recb = recp.tile([64, 2, N], f32, tag="recb")
                    nc.gpsimd.partition_broadcast(recb, rec)
                    for par in (0, 1):
                        nc.vector.tensor_mul(
                            out=o_all[par * 64:par * 64 + 64, hp, b, :],
                            in0=ps_o[0:64, par, :],
                            in1=recb[:, par, :],
                        )

                def make_proj_unit(et, o_all_=o_all, b0_=b0):
                    def unit():
                        psp = psbig.tile([128, 2, N], f32, tag="mmbig",
                                         name=f"psp{et}")
                        for hp in range(6):
                            nc.tensor.matmul(
                                psp,
                                projw_sb[:, hp, et * 128:(et + 1) * 128],
                                o_all_[:, hp],
                                start=(hp == 0),
                                stop=(hp == 5),
                            )
                        ob = obp.tile([128, 2, N], f32, tag="ob", name=f"ob{et}")
                        nc.scalar.add(ob, psp, projb_sb[:, et:et + 1])
                        nc.sync.dma_start(
                            outT[b0_:b0_ + 2, et * 128:(et + 1) * 128, :]
                            .rearrange("b p n -> p b n"),
                            ob,
                        )
                    return unit

                work = [(b, hp) for b in (0, 1) for hp in range(6)]
                pending = []
                for i, (b, hp) in enumerate(work):
                    pending.append((b, hp, emit_scores(b, hp)))
                    if prev_proj_units and i % 2 == 1:
                        prev_proj_units.pop(0)()
                    # spread next pair's 20 qkv chunks over units 2..11;
                    # delayed past unit 2 so the ACT queue doesn't head-of-
                    # line block on v-copies waiting for prev-pair AV reads
                    if i >= 2:
                        for _ in range(2):
                            if fillers:
                                fillers.pop(0)()
                    if len(pending) > pdepth:
                        pb, php, ppus = pending.pop(0)
                        emit_av(pb, php, ppus)
                for pb, php, ppus in pending:
                    emit_av(pb, php, ppus)
                while fillers:
                    fillers.pop(0)()
                for u in prev_proj_units:
                    u()
                prev_proj_units = [make_proj_unit(et) for et in range(6)]
                xt, qk = next_xt, next_qk

            # ---- final pair's proj ----
            for u in prev_proj_units:
                u()
    nc.compile()
    return nc


def _to_bf16(a):
    import ml_dtypes

    return np.asarray(a, np.float32).astype(ml_dtypes.bfloat16)


def prep_inputs(x, qkv_w, proj_w, proj_b, bias_table, rel_idx):
    """Host-side data prep shared by kernel() and test harness."""
    x = np.asarray(x, np.float32)
    qkv_w = np.asarray(qkv_w, np.float32)
    proj_w = np.asarray(proj_w, np.float32)
    proj_b = np.asarray(proj_b, np.float32)
    bias_table = np.asarray(bias_table, np.float32)
    rel_idx = np.asarray(rel_idx)

    xT = np.ascontiguousarray(x.reshape(NCORES, BLOC, N, C).transpose(0, 1, 3, 2))
    qkv_wT = np.ascontiguousarray(qkv_w.T)
    proj_wT = np.ascontiguousarray(proj_w.T)
    proj_bt = np.ascontiguousarray(proj_b.reshape(6, 128).T)
    bias_full = bias_table[rel_idx]  # [n, m, h]
    # biasT[m, h, n] = exp(bias[n, m, h])
    biasT = np.ascontiguousarray(np.exp(bias_full).transpose(1, 2, 0))
    return (
        _to_bf16(xT), _to_bf16(qkv_wT), _to_bf16(proj_wT),
        proj_bt, _to_bf16(biasT),
    )


_NC_CACHE = {}


def _get_nc(**kw):
    key = tuple(sorted(kw.items()))
    if key not in _NC_CACHE:
        _NC_CACHE[key] = build_nc(**kw)
    return _NC_CACHE[key]


def kernel(x, qkv_w, proj_w, proj_b, bias_table, rel_idx, _trace=False):
    from concourse.bass_utils import run_bass_kernel_spmd

    xT, qkv_wT, proj_wT, proj_bt, biasT = prep_inputs(
        x, qkv_w, proj_w, proj_b, bias_table, rel_idx
    )
    nc = _get_nc()
    in_maps = [
        {
            "xT": np.ascontiguousarray(xT[c]),
            "qkv_wT": qkv_wT,
            "proj_wT": proj_wT,
            "proj_bt": proj_bt,
            "biasT": biasT,
        }
        for c in range(NCORES)
    ]
    res = run_bass_kernel_spmd(nc, in_maps, list(range(NCORES)), trace=_trace)
    outs = np.stack([res.results[c]["outT"] for c in range(NCORES)])
    out = outs.reshape(B, C, N).transpose(0, 2, 1)
    out = np.ascontiguousarray(out, np.float32)
    if _trace:
        return out, res
    return out


# revision 18
# speedup vs baseline: 1.4719x; 1.1446x over previous
"""Trainium2 Bass kernel for Swin-style multi-head attention (bf16 v2).

Problem: x[128,197,768] -> qkv -> 12-head attention with relative-position
bias -> proj. Data-parallel over batch across 8 NeuronCores (16 batches/core).

v2 design vs the fp32r baseline:
  - All matmul operands are bf16 (1 cyc/col, same as fp32r, but with no
    >=256 free-dim requirement), so the n dimension is NOT padded to 256:
    scores/AV/exp/bias-mul process 197-wide tiles (-23% columns there).
  - Host ships xT / qkv_wT / proj_wT / exp(bias) already in bf16: DMA
    traffic halves and every fp32r staging copy (ACT+GPSIMD pairs in the
    baseline) collapses into one direct ACT copy PSUM->bf16 SBUF.
  - Epilogue balance: ACT does exp, qk/v copies and the proj bias-add
    (Identity activation + per-partition bias AP); DVE does the bias
    multiply (2x bf16 mode), reciprocal and the normalize multiply;
    GPSIMD only does one merged partition_broadcast per unit.
  - vaug tiles ([t, 12, 65] with a persistent ones column for the
    augmented-matmul rowsum trick) are allocated per pair-parity and the
    ones column is initialized once, outside the pair loop.
"""

import sys

import numpy as np

for _p in ('/opt/trn_rl_repo', '/root/.axon_site/_ro/trn_rl_repo'):
    if _p not in sys.path:
        sys.path.insert(0, _p)

B = 128
N = 197
C = 768
H = 12
DH = 64
SCALE = DH ** -0.5
NCORES = 8
BLOC = B // NCORES  # 16
M0, M1 = 128, N - 128  # key-dim tiles: 128 + 69


def build_nc(b_loc=BLOC, pdepth=4, reps=1):
    import concourse.bacc as bacc
    import concourse.tile as tile
    from concourse import library_config, mybir

    f32 = mybir.dt.float32
    bf16 = mybir.dt.bfloat16

    nc = bacc.Bacc("TRN2", target_bir_lowering=False, debug=False)
    xT = nc.dram_tensor("xT", [b_loc, C, N], bf16, kind="ExternalInput").ap()
    qkv_wT = nc.dram_tensor("qkv_wT", [C, 3 * C], bf16, kind="ExternalInput").ap()
    proj_wT = nc.dram_tensor("proj_wT", [C, C], bf16, kind="ExternalInput").ap()
    proj_bt = nc.dram_tensor("proj_bt", [128, 6], f32, kind="ExternalInput").ap()
    biasT = nc.dram_tensor("biasT", [N, H, N], bf16, kind="ExternalInput").ap()
    outT = nc.dram_tensor("outT", [b_loc, C, N], f32, kind="ExternalOutput").ap()

    n_pairs = b_loc // 2

    with tile.TileContext(nc) as tc:
        with (
            tc.tile_pool(name="consts", bufs=1) as consts,
            tc.tile_pool(name="xtp", bufs=2) as xtp,
            tc.tile_pool(name="qkp", bufs=2) as qkp,
            tc.tile_pool(name="qoddp", bufs=2) as qoddp,
            tc.tile_pool(name="sap", bufs=4) as sap,
            tc.tile_pool(name="vstp", bufs=2) as vstp,
            tc.tile_pool(name="pup", bufs=2 * (pdepth + 1)) as pup,
            tc.tile_pool(name="recp", bufs=3) as recp,
            tc.tile_pool(name="oallp", bufs=2) as oallp,
            tc.tile_pool(name="obp", bufs=6) as obp,
            tc.tile_pool(name="psbig", bufs=2, space="PSUM") as psbig,
            tc.tile_pool(name="pss", bufs=2, space="PSUM") as pss,
            tc.tile_pool(name="pso", bufs=2, space="PSUM") as pso,
        ):
            nc.gpsimd.load_library(library_config.attnmlp)

            qkvw_sb = consts.tile([128, 6, 3 * C], bf16)
            projw_sb = consts.tile([128, 6, C], bf16)
            projb_sb = consts.tile([128, 6], f32)
            bias0_sb = consts.tile([128, H, N], bf16)
            bias1_sb = consts.tile([128, H, N], bf16)
            bias_sb = (bias0_sb, bias1_sb)
            # vaug tiles [t, 12, 65]: per parity x batch-in-pair x t-chunk.
            # Column 64 is a persistent 1.0 (rowsum trick), set once below;
            # the v copies never touch it.
            vts_all = [
                [
                    [
                        consts.tile([128, H, 65], bf16, name=f"vt{par}{b}{tci}")
                        for tci in (0, 1)
                    ]
                    for b in (0, 1)
                ]
                for par in (0, 1)
            ]

            # split the big weight load across queues; qkv first (needed by
            # the first pair's matmuls), proj weights last (needed ~30us in)
            for ct in range(6):
                nc.sync.dma_start(
                    qkvw_sb[:, ct], qkv_wT[ct * 128:(ct + 1) * 128, :]
                )
            nc.sync.dma_start(projb_sb, proj_bt)
            nc.sync.dma_start(bias0_sb, biasT[0:M0])
            nc.sync.dma_start(bias1_sb[:M1], biasT[M0:N])
            nc.sync.dma_start(
                projw_sb, proj_wT.rearrange("(hp p) e -> p hp e", p=128)
            )
            ones_sb = consts.tile([128, H], bf16)
            nc.vector.memset(ones_sb, 1.0)
            for par in (0, 1):
                for b in (0, 1):
                    for tci in (0, 1):
                        nc.gpsimd.tensor_copy(
                            out=vts_all[par][b][tci][:, :, 64], in_=ones_sb
                        )

            def load_xt(p):
                xt = xtp.tile([128, 6, 2, N], bf16)
                for b in (0, 1):
                    nc.sync.dma_start(
                        xt[:, :, b, :],
                        xT[2 * (p % n_pairs) + b].rearrange(
                            "(ct p) n -> p ct n", p=128
                        ),
                    )
                return xt

            def make_qkv_units(xt_, qs_, qodd_, vts_):
                """20 PE-burst closures (12 qk f-tiles + 8 v chunks) plus 12
                deferred odd-head shift copies. The qk PSUM is drained by one
                full-height ACT copy into qs_ (rows 0:64 = even head, directly
                usable at base partition 0); the odd half is moved to qodd_ by
                a 2x-rate SBUF->SBUF DVE copy emitted later, off the
                attention-critical DVE windows."""
                units = []
                odd_copies = []
                for ft in range(12):
                    def qk_unit(ft=ft):
                        ps = psbig.tile([128, 2, N], f32, tag="mmbig")
                        for ct in range(6):
                            nc.tensor.matmul(
                                ps,
                                qkvw_sb[:, ct, ft * 128:(ft + 1) * 128],
                                xt_[:, ct],
                                start=(ct == 0),
                                stop=(ct == 5),
                            )
                        nc.scalar.copy(out=qs_[:, ft], in_=ps)
                    units.append(qk_unit)
                    def odd_copy(ft=ft):
                        nc.vector.tensor_copy(
                            out=qodd_[:, ft], in_=qs_[64:128, ft]
                        )
                    odd_copies.append(odd_copy)
                for b in (0, 1):
                    for tci, (t0, tsz) in enumerate(((0, M0), (M0, M1))):
                        for half in (0, 1):
                            def v_unit(b=b, tci=tci, t0=t0, tsz=tsz, half=half):
                                vt = vts_[b][tci]
                                vt_r = vt.rearrange(
                                    "p (g two) c -> p two g c", two=2
                                )
                                psv = psbig.tile([128, 384], f32, tag="mmbig")
                                for ct in range(6):
                                    nc.tensor.matmul(
                                        psv[:tsz],
                                        xt_[:, ct, b, t0:t0 + tsz],
                                        qkvw_sb[:, ct,
                                                2 * C + half * 384:
                                                2 * C + (half + 1) * 384],
                                        start=(ct == 0),
                                        stop=(ct == 5),
                                    )
                                nc.scalar.copy(
                                    out=vt_r[:tsz, :,
                                             half * 3:(half + 1) * 3, 0:64]
                                    .rearrange("p two g d -> p g two d"),
                                    in_=psv[:tsz].rearrange(
                                        "p (g two d) -> p g two d", g=3, two=2
                                    ),
                                )
                            units.append(v_unit)
                # order odd copies by first consumer: unit hp needs (hp, 6+hp)
                ordered = []
                for hp in range(6):
                    ordered += [odd_copies[hp], odd_copies[6 + hp]]
                return units, ordered

            prev_proj_units = []
            total_pairs = reps * n_pairs
            # pair 0's qkv runs un-interleaved (prologue)
            xt = load_xt(0)
            qs = qkp.tile([128, H, 2, N], bf16)
            qodd = qoddp.tile([64, H, 2, N], bf16)
            units0, odds0 = make_qkv_units(xt, qs, qodd, vts_all[0])
            for u in units0 + odds0:
                u()
            for pp in range(total_pairs):
                b0 = 2 * (pp % n_pairs)
                parity = pp % 2
                vts = vts_all[parity]

                # next pair's inputs + qkv filler closures, interleaved into
                # this pair's attention so the PE never starves at the tail
                if pp + 1 < total_pairs:
                    next_xt = load_xt(pp + 1)
                    next_qs = qkp.tile([128, H, 2, N], bf16)
                    next_qodd = qoddp.tile([64, H, 2, N], bf16)
                    fillers, odd_copies = make_qkv_units(
                        next_xt, next_qs, next_qodd, vts_all[(pp + 1) % 2]
                    )
                else:
                    next_xt = next_qs = next_qodd = None
                    fillers, odd_copies = [], []

                # ---- attention, software-pipelined over (batch, head-pair) ----
                o_all = oallp.tile([128, 6, 2, N], bf16)

                def emit_scores(b, hp):
                    h0 = 2 * hp
                    q0 = qs[0:64, hp, b, :]
                    k0 = qs[0:64, 6 + hp, b, :]
                    q1 = qodd[:, hp, b, :]
                    k1 = qodd[:, 6 + hp, b, :]
                    pus = []
                    for mt, (m0, msz) in enumerate(((0, M0), (M0, M1))):
                        ps_s = pss.tile([128, 2, N], f32, tag=f"s{mt}")
                        nc.tensor.matmul(
                            ps_s[:msz, 0, :], k0[:, m0:m0 + msz], q0,
                            start=True, stop=True,
                        )
                        nc.tensor.matmul(
                            ps_s[:msz, 1, :], k1[:, m0:m0 + msz], q1,
                            start=True, stop=True,
                        )
                        sa = sap.tile([128, 2, N], bf16, tag=f"sa{mt}")
                        nc.scalar.activation(
                            out=sa[:msz], in_=ps_s[:msz],
                            func=mybir.ActivationFunctionType.Exp, scale=SCALE,
                        )
                        pu = pup.tile([128, 2, N], bf16, tag=f"pu{mt}")
                        nc.vector.tensor_mul(
                            out=pu[:msz], in0=sa[:msz],
                            in1=bias_sb[mt][:msz, h0:h0 + 2, :],
                        )
                        pus.append(pu)
                    return pus

                def emit_av(b, hp, pus):
                    h0, h1 = 2 * hp, 2 * hp + 1
                    vt0, vt1 = vts[b]
                    ps_o = pso.tile([128, 2, N], f32, tag="opair")
                    for par, h in ((0, h0), (1, h1)):
                        nc.tensor.matmul(
                            ps_o[0:65, par, :], vt0[:, h, :],
                            pus[0][:, par, :], start=True, stop=False,
                        )
                        nc.tensor.matmul(
                            ps_o[0:65, par, :], vt1[:M1, h, :],
                            pus[1][:M1, par, :], start=False, stop=True,
                        )
                    rec = recp.tile([1, 2, N], f32, tag="rec")
                    nc.vector.reciprocal(out=rec, in_=ps_o[64:65, :, :])
                    recb = recp.tile([64, 2, N], f32, tag="recb")
                    nc.gpsimd.partition_broadcast(recb, rec)
                    for par in (0, 1):
                        nc.vector.tensor_mul(
                            out=o_all[par * 64:par * 64 + 64, hp, b, :],
                            in0=ps_o[0:64, par, :],
                            in1=recb[:, par, :],
                        )

                def make_proj_unit(et, o_all_=o_all, b0_=b0):
                    def unit():
                        psp = psbig.tile([128, 2, N], f32, tag="mmbig",
                                         name=f"psp{et}")
                        for hp in range(6):
                            nc.tensor.matmul(
                                psp,
                                projw_sb[:, hp, et * 128:(et + 1) * 128],
                                o_all_[:, hp],
                                start=(hp == 0),
                                stop=(hp == 5),
                            )
                        ob = obp.tile([128, 2, N], f32, tag="ob", name=f"ob{et}")
                        nc.scalar.add(ob, psp, projb_sb[:, et:et + 1])
                        nc.sync.dma_start(
                            outT[b0_:b0_ + 2, et * 128:(et + 1) * 128, :]
                            .rearrange("b p n -> p b n"),
                            ob,
                        )
                    return unit

                work = [(b, hp) for b in (0, 1) for hp in range(6)]
                pending = []
                for i, (b, hp) in enumerate(work):
                    pending.append((b, hp, emit_scores(b, hp)))
                    if prev_proj_units and i % 2 == 1:
                        prev_proj_units.pop(0)()
                    # spread next pair's 20 qkv chunks over units 2..11;
                    # delayed past unit 2 so the ACT queue doesn't head-of-
                    # line block on v-copies waiting for prev-pair AV reads
                    if i >= 2:
                        for _ in range(2):
                            if fillers:
                                fillers.pop(0)()
                    if len(pending) > pdepth:
                        pb, php, ppus = pending.pop(0)
                        emit_av(pb, php, ppus)
                for pb, php, ppus in pending:
                    emit_av(pb, php, ppus)
                while fillers:
                    fillers.pop(0)()
                # deferred odd-head shift copies for the next pair, emitted
                # outside the attention-critical DVE windows
                for oc in odd_copies:
                    oc()
                for u in prev_proj_units:
                    u()
                prev_proj_units = [make_proj_unit(et) for et in range(6)]
                xt, qs, qodd = next_xt, next_qs, next_qodd

            # ---- final pair's proj ----
            for u in prev_proj_units:
                u()
    nc.compile()
    return nc


def _to_bf16(a):
    import ml_dtypes

    return np.asarray(a, np.float32).astype(ml_dtypes.bfloat16)


def prep_inputs(x, qkv_w, proj_w, proj_b, bias_table, rel_idx):
    """Host-side data prep shared by kernel() and test harness."""
    x = np.asarray(x, np.float32)
    qkv_w = np.asarray(qkv_w, np.float32)
    proj_w = np.asarray(proj_w, np.float32)
    proj_b = np.asarray(proj_b, np.float32)
    bias_table = np.asarray(bias_table, np.float32)
    rel_idx = np.asarray(rel_idx)

    xT = np.ascontiguousarray(x.reshape(NCORES, BLOC, N, C).transpose(0, 1, 3, 2))
    qkv_wT = np.ascontiguousarray(qkv_w.T)
    proj_wT = np.ascontiguousarray(proj_w.T)
    proj_bt = np.ascontiguousarray(proj_b.reshape(6, 128).T)
    bias_full = bias_table[rel_idx]  # [n, m, h]
    # biasT[m, h, n] = exp(bias[n, m, h])
    biasT = np.ascontiguousarray(np.exp(bias_full).transpose(1, 2, 0))
    return (
        _to_bf16(xT), _to_bf16(qkv_wT), _to_bf16(proj_wT),
        proj_bt, _to_bf16(biasT),
    )


_NC_CACHE = {}


def _get_nc(**kw):
    key = tuple(sorted(kw.items()))
    if key not in _NC_CACHE:
        _NC_CACHE[key] = build_nc(**kw)
    return _NC_CACHE[key]


def kernel(x, qkv_w, proj_w, proj_b, bias_table, rel_idx, _trace=False):
    from concourse.bass_utils import run_bass_kernel_spmd

    xT, qkv_wT, proj_wT, proj_bt, biasT = prep_inputs(
        x, qkv_w, proj_w, proj_b, bias_table, rel_idx
    )
    nc = _get_nc()
    in_maps = [
        {
            "xT": np.ascontiguousarray(xT[c]),
            "qkv_wT": qkv_wT,
            "proj_wT": proj_wT,
            "proj_bt": proj_bt,
            "biasT": biasT,
        }
        for c in range(NCORES)
    ]
    res = run_bass_kernel_spmd(nc, in_maps, list(range(NCORES)), trace=_trace)
    outs = np.stack([res.results[c]["outT"] for c in range(NCORES)])
    out = outs.reshape(B, C, N).transpose(0, 2, 1)
    out = np.ascontiguousarray(out, np.float32)
    if _trace:
        return out, res
    return out
